# revision 1
# baseline (speedup 1.0000x reference)
"""Trainium2 Bass kernel for nn_Att_patch_net (vq_codebook).

Math (per batch b):
  y[d,pix]   = relu(BN(conv1x1(x)))                    -- folded into Wc, bias_d
  dist[k,pix]= ||y[:,pix]-c[k]||^2 ; A = softmax_k(scale_k*dist)   (per PIXEL)
  R[pix]     = A[pix,k]*(y[d,pix]-c[k,d])  in R^{K x D}
  E[p]       = sum_{pix in patch p} R[pix]             -- 5x5 box sums (100 patches)
  v          = sum_p E[p]/||E[p]||  ;  logits = v @ cls_w.T + cls_b

Device strategy (8 cores, 4 batches each):
  * A[pix] depends only on the pixel -> per-pixel softmax via matmuls.
    u = x2 - dist = 2<y,c> - c2 computed DIRECTLY (no cancellation).
  * ||E[p]||^2 = sum_{pix,pix' in p} <R_pix,R_pix'> with the Gram matrix
      G = YY*S1 - (A'At2 + At2'A)/2    (bf16 matmuls, no R materialized)
    n2 = diag(M G M^T) via T' = (M G)^T pieces + fused mul-reduce.
  * v = sum_pix w[pix]*R[pix] with w = M^T (1/||E||)   (transposed box filter)
  * classifier: class-sharded (125/core) over an AllGather of v (bf16).

Precision: inputs/weights cast to bf16 host-side; PE accumulates fp32;
invn = exp(-0.5*ln(n2)) keeps every Act func in one act-table set.
"""
import numpy as np
from contextlib import ExitStack

B, CIN, HH, WW = 32, 2048, 14, 14
D, K, NCLASS = 128, 32, 1000
WIN = 5
PH = PW = 10
NP_ = 100          # patches
NPIX = HH * WW     # 196
PPIX = 256         # padded pixels per batch slot (gram layout)
NCORES = 8
BL = B // NCORES   # 4 local batches
CSH = NCLASS // NCORES  # 125 classes per core
NCH = CIN // 128   # 16 contraction chunks
NGRP = 4           # x DMA groups (NCH/NGRP chunks each)
BN_EPS = 1e-5
UNPIX = BL * NPIX  # 784 unpadded pixel columns

# packed bf16 const columns
PKB_CW2 = 0          # [128, 32]
PKB_C2T = 32         # [128, 32] 2*codewords^T (contiguous with CW2)
PKB_SC = 64          # [128, 32] scale bcast
PKB_ID = 96          # [128, 128] identity (bf16)
PKB_ONE = 224        # [128, 32] ones
PKB_MMT = 256        # [128, 200] M^T chunks (2x100)
PKB_MMTH = 456       # [128, 200] -0.5*M^T chunks
PKB_MM = 656         # [128, 256] M (rows 0:100)
PKB_BMH = 912        # [0:4, 128] bmh rows
PKB_BMH3 = 1040      # [0:1, 128] bmh row 3 at partition 0
PKB_N = 1168
# packed f32 const columns
PKF_BIAS = 0
PKF_EB = 1
PKF_NC2 = 2
PKF_CT = 3           # [128, 128] codewords tiled
PKF_BM = 131         # [0:4, 128] bm rows
PKF_CLSB = 259       # [0:125, 1]
PKF_ONE = 260        # [128, 32] ones (f32)
PKF_N = 292

_NC_CACHE = {}
TTR = False
STAGE = 4           # 1=conv+softmax, 2=+gram/norms, 3=+v (no collective), 4=full
WRITE_V = False     # add a 'vout' external output with v^T (2-launch fallback)


# ----------------------------------------------------------------- device IR
def build_nc(n_rep: int = 1):
    import concourse.bass as bass
    import concourse.tile as tile
    from concourse import bacc, mybir

    f32 = mybir.dt.float32
    bf16 = mybir.dt.bfloat16
    AF = mybir.ActivationFunctionType
    OP = mybir.AluOpType
    AX = mybir.AxisListType

    nc = bacc.Bacc("TRN2", target_bir_lowering=False, debug=False, num_devices=NCORES)

    def din(name, shape, dt=f32):
        return nc.dram_tensor(name, shape, dt, kind="ExternalInput").ap()

    xin = din("xin", [BL, 128, NCH * NPIX], bf16)
    wct = din("wct", [128, NCH * 128], bf16)
    pkb = din("pkb", [128, PKB_N], bf16)
    pkf = din("pkf", [128, PKF_N], f32)
    clsw = din("clsw", [128, K * CSH], bf16)
    logits = nc.dram_tensor("logits", [CSH, B], f32, kind="ExternalOutput").ap()
    vout = (nc.dram_tensor("vout", [128, 128], bf16, kind="ExternalOutput").ap()
            if WRITE_V else None)

    with tile.TileContext(nc) as tc, ExitStack() as ctx:
        cp = ctx.enter_context(tc.tile_pool(name="consts", bufs=1))
        xp = ctx.enter_context(tc.tile_pool(name="xp", bufs=4))
        yp = ctx.enter_context(tc.tile_pool(name="yp", bufs=2))
        ap_ = ctx.enter_context(tc.tile_pool(name="ap", bufs=2))
        sp = ctx.enter_context(tc.tile_pool(name="sp", bufs=2))
        gp = ctx.enter_context(tc.tile_pool(name="gp", bufs=2))
        ytp = ctx.enter_context(tc.tile_pool(name="ytp", bufs=10))
        ps = ctx.enter_context(tc.tile_pool(name="ps", bufs=1, space="PSUM"))
        dp = ctx.enter_context(tc.tile_pool(name="dp", bufs=2, space="DRAM"))

        # PE p-state warmup: keep PE busy from t=0 so the conv starts at
        # full clock. Zeros tile via memset (no DMA dependency); dead output.
        wz = cp.tile([128, 128], bf16)
        nc.vector.memset(wz[:], 0.0)
        wup = ps.tile([128, 128], bf16, tag="B", bufs=2, name="warm")
        for _i in range(44):
            nc.tensor.transpose(wup[:], wz[:], wz[:])

        # ---- prologue DMAs, ordered for launch-critical-path overlap ----
        def issue_x(rep, b):
            xt = xp.tile([128, NCH, NPIX], bf16, tag="xg",
                         name=f"xt_{rep}_{b}", bufs=4)
            xv = xin[b].rearrange("p (n m) -> p n m", n=NCH)
            h = NCH // 2
            nc.sync.dma_start(out=xt[:, 0:h, :], in_=xv[:, 0:h, :])
            nc.sync.dma_start(out=xt[:, h:NCH, :], in_=xv[:, h:NCH, :])
            return xt

        wc_sb = cp.tile([128, NCH, 128], bf16)
        wcv = wct.rearrange("p (n m) -> p n m", n=NCH)
        nc.sync.dma_start(out=wc_sb[:, 0:NCH // 2, :], in_=wcv[:, 0:NCH // 2, :])
        x0 = [issue_x(0, 0)]
        nc.sync.dma_start(out=wc_sb[:, NCH // 2:NCH, :],
                          in_=wcv[:, NCH // 2:NCH, :])
        pkb_sb = cp.tile([128, PKB_N], bf16)
        nc.sync.dma_start(out=pkb_sb[:], in_=pkb[:])
        pkf_sb = cp.tile([128, PKF_N], f32)
        nc.sync.dma_start(out=pkf_sb[:], in_=pkf[:])
        for _b in range(1, BL):
            x0.append(issue_x(0, _b))
        if STAGE >= 4:
            clsw_sb = cp.tile([128, K, CSH], bf16)
            nc.sync.dma_start(
                out=clsw_sb[:], in_=clsw.rearrange("p (k n) -> p k n", k=K))

        cw2T = pkb_sb[:, PKB_CW2:PKB_CW2 + K]
        scbc = pkb_sb[:, PKB_SC:PKB_SC + K]
        czT = pkb_sb[:, PKB_CW2:PKB_CW2 + 2 * K]
        identb = pkb_sb[:, PKB_ID:PKB_ID + 128]
        onesq = pkb_sb[0:32, PKB_ONE:PKB_ONE + 32]
        mmT = pkb_sb[:, PKB_MMT:PKB_MMT + 2 * NP_].rearrange(
            "p (q n) -> p q n", q=2)
        mmTh = pkb_sb[:, PKB_MMTH:PKB_MMTH + 2 * NP_].rearrange(
            "p (q n) -> p q n", q=2)
        mmat = pkb_sb[0:NP_, PKB_MM:PKB_MM + PPIX]
        bmh_sb = pkb_sb[0:BL, PKB_BMH:PKB_BMH + 128]
        bmh3_sb = pkb_sb[0:1, PKB_BMH3:PKB_BMH3 + 128]
        biasd = pkf_sb[:, PKF_BIAS:PKF_BIAS + 1]
        ebias = pkf_sb[0:32, PKF_EB:PKF_EB + 1]
        negc2 = pkf_sb[0:32, PKF_NC2:PKF_NC2 + 1]
        ctile = pkf_sb[:, PKF_CT:PKF_CT + 128]
        clsb_sb = pkf_sb[0:CSH, PKF_CLSB:PKF_CLSB + 1]

        for _rep in range(n_rep):
            y_bf = yp.tile([128, BL, PPIX], bf16, tag="y", name=f"y_{_rep}")
            nc.vector.memset(y_bf[:, :, NPIX:PPIX], 0.0)
            a_tF = ap_.tile([128, PPIX], bf16, tag="at", name=f"at_{_rep}")
            nc.vector.memset(a_tF[:, NPIX:PPIX], 0.0)
            at2F = ap_.tile([128, PPIX], bf16, tag="at2", name=f"at2_{_rep}")
            nc.vector.memset(at2F[:, NPIX:PPIX], 0.0)
            u_sbF = sp.tile([128, NPIX], f32, tag="u", name=f"u_{_rep}")
            if STAGE >= 2:
                n2_sb = sp.tile([NP_, BL], f32, tag="n2", name=f"n2_{_rep}")
            yts = []

            def emit_front(b):
                # ============ conv + BN + ReLU (batch b) ============
                xt = x0[b] if _rep == 0 else issue_x(_rep, b)
                pc = ps.tile([128, NPIX], f32, tag="C", bufs=2,
                             name=f"pc_{_rep}_{b}", padded_shape=[128, 512])
                for k in range(NCH):
                    nc.tensor.matmul(pc[:], wc_sb[:, k, :], xt[:, k, :],
                                     start=(k == 0), stop=(k == NCH - 1))
                nc.scalar.activation(y_bf[:, b, 0:NPIX], pc[:], AF.Relu,
                                     bias=biasd)
                ysq_b = sp.tile([128, NPIX], bf16, tag="ysq", bufs=3,
                                name=f"ysq_{_rep}_{b}")
                nc.gpsimd.tensor_mul(ysq_b[:], y_bf[:, b, 0:NPIX],
                                     y_bf[:, b, 0:NPIX])

                # ============ per-pixel softmax (batch b) ============
                zu = ps.tile([64, NPIX], f32, tag="B", bufs=2,
                             name=f"zu_{_rep}_{b}", padded_shape=[64, 512])
                zb = zu[0:32, :]
                ub = zu[32:64, :]
                nc.tensor.matmul(zu[:], czT, y_bf[:, b, 0:NPIX], start=True,
                                 stop=False, skip_group_check=True)
                nc.tensor.matmul(zb, scbc, ysq_b[:], start=False, stop=True,
                                 skip_group_check=True)
                a_u = ap_.tile([32, NPIX], bf16, tag="au", bufs=3,
                               name=f"au_{_rep}_{b}")
                nc.scalar.activation(a_u[:], zb, AF.Exp, bias=ebias)
                u_sb = u_sbF[32 * b:32 * b + 32, :]
                nc.scalar.activation(u_sb[:], ub, AF.Identity, bias=negc2)
                # replicated-sum: all-ones [32,32] stationary gives the
                # softmax denominator already broadcast across partitions
                s_ps = ps.tile([32, NPIX], f32, tag="B", bufs=2,
                               name=f"s_{_rep}_{b}", padded_shape=[32, 512])
                nc.tensor.matmul(s_ps[:], onesq, a_u[:], start=True, stop=True)
                s_r = sp.tile([32, NPIX], f32, tag="sr", bufs=2,
                              name=f"sr_{_rep}_{b}")
                nc.vector.reciprocal(s_r[:], s_ps[:])
                asl = a_tF[32 * b:32 * b + 32, :]
                a2sl = at2F[32 * b:32 * b + 32, :]
                nc.vector.tensor_mul(asl[:, 0:NPIX], a_u[:], s_r[:])
                nc.gpsimd.tensor_mul(a2sl[:, 0:NPIX], u_sb[:], asl[:, 0:NPIX])
                if STAGE >= 3:
                    # y^T tiles for the final aggregation
                    ytb = []
                    for q in range(2):
                        ytq_ps = ps.tile([128, 128], bf16, tag="B", bufs=2,
                                         name=f"ytps_{_rep}_{b}_{q}")
                        nc.tensor.transpose(ytq_ps[:],
                                            y_bf[:, b, 128 * q:128 * q + 128],
                                            identb)
                        ytq = ytp.tile([128, 128], bf16, tag="yt",
                                       name=f"yt_{_rep}_{b}_{q}")
                        if q == 0:
                            nc.scalar.copy(ytq[:], ytq_ps[:])
                        else:
                            nc.vector.tensor_copy(ytq[:], ytq_ps[:])
                        ytb.append(ytq)
                    yts.append(ytb)

            def emit_gram(b):
                asl = a_tF[32 * b:32 * b + 32, :]
                a2sl = at2F[32 * b:32 * b + 32, :]
                if STAGE >= 2:
                    # ============ Gram / patch norms (batch b) ============
                    g1s = []
                    wcs = []
                    for q in range(2):
                        yy_ps = ps.tile([128, PPIX], f32, tag="D", bufs=4,
                                        name=f"yy_{_rep}_{b}_{q}")
                        nc.tensor.matmul(yy_ps[:],
                                         y_bf[:, b, 128 * q:128 * q + 128],
                                         y_bf[:, b, :], start=True, stop=True)
                        s1_ps = ps.tile([128, PPIX], f32, tag="D", bufs=4,
                                        name=f"s1_{_rep}_{b}_{q}")
                        nc.tensor.matmul(s1_ps[:], asl[:, 128 * q:128 * q + 128],
                                         asl[:], start=True, stop=True,
                                         tile_position=(32 * b, 0))
                        w_ps = ps.tile([128, PPIX], f32, tag="D", bufs=4,
                                       name=f"w_{_rep}_{b}_{q}")
                        nc.tensor.matmul(w_ps[:], a2sl[:, 128 * q:128 * q + 128],
                                         asl[:], start=True, stop=False,
                                         tile_position=(32 * b, 0))
                        nc.tensor.matmul(w_ps[:], asl[:, 128 * q:128 * q + 128],
                                         a2sl[:], start=False, stop=True,
                                         tile_position=(32 * b, 0))
                        s1c = gp.tile([128, PPIX], bf16, tag=f"s1c{q}", bufs=3,
                                      name=f"s1c_{_rep}_{b}_{q}")
                        nc.scalar.copy(s1c[:], s1_ps[:])
                        g1 = gp.tile([128, PPIX], bf16, tag=f"g1{q}", bufs=3,
                                     name=f"g1_{_rep}_{b}_{q}")
                        nc.vector.tensor_mul(g1[:], yy_ps[:], s1c[:])
                        wc = gp.tile([128, PPIX], bf16, tag=f"wc{q}", bufs=3,
                                     name=f"wc_{_rep}_{b}_{q}")
                        if q == 0:
                            nc.scalar.copy(wc[:], w_ps[:])
                        else:
                            nc.vector.tensor_copy(wc[:], w_ps[:])
                        g1s.append(g1)
                        wcs.append(wc)
                    # T' = (M G)^T : [100, 256] psum, then n2 = rowsum(T'*M)
                    tp_ps = ps.tile([NP_, PPIX], f32, tag="D", bufs=4,
                                    name=f"tp_{_rep}_{b}")
                    for q in range(2):
                        nc.tensor.matmul(tp_ps[:], mmT[:, q, :], g1s[q][:],
                                         start=(q == 0), stop=False)
                        nc.tensor.matmul(tp_ps[:], mmTh[:, q, :], wcs[q][:],
                                         start=False, stop=(q == 1))
                    tm = sp.tile([NP_, PPIX], bf16, tag="tm",
                                 name=f"tm_{_rep}_{b}")
                    nc.vector.scalar_tensor_tensor(
                        out=tm[:], in0=tp_ps[:], scalar=1.0, in1=mmat,
                        op0=OP.mult, op1=OP.mult,
                        accum_out=n2_sb[:, b:b + 1])

            # software-pipelined emission: batch b's gram is emitted after
            # batch b+1's softmax so ready work never queues behind
            # not-yet-ready work on the in-order engines.
            for b in range(BL):
                emit_front(b)
                if b >= 1:
                    emit_gram(b - 1)
            emit_gram(BL - 1)

            if STAGE >= 3:
                # ============ patch weights: invn = |n2|^(-1/2) ========
                # batches 0-2 computed early (overlaps batch 3's gram);
                # batch 3 joins at the end. Both ars ops sit after the last
                # Exp on the Act queue so the act-table loads stay at 2.
                invn = sp.tile([NP_, BL], bf16, tag="invn", name=f"invn_{_rep}")
                nc.scalar.activation(invn[:, 0:BL - 1], n2_sb[:, 0:BL - 1],
                                     AF.Abs_reciprocal_sqrt)
                w_ps2 = ps.tile([BL, PPIX], f32, tag="B", bufs=2,
                                name=f"wps_{_rep}")
                nc.tensor.matmul(w_ps2[0:BL - 1, :], invn[:, 0:BL - 1], mmat,
                                 start=True, stop=True)
                w_sb = sp.tile([BL, PPIX], bf16, tag="wsb", name=f"wsb_{_rep}")
                nc.scalar.copy(w_sb[0:BL - 1, :], w_ps2[0:BL - 1, :])
                nc.scalar.activation(invn[:, BL - 1:BL], n2_sb[:, BL - 1:BL],
                                     AF.Abs_reciprocal_sqrt)
                w3_ps = ps.tile([1, PPIX], f32, tag="B", bufs=2,
                                name=f"w3ps_{_rep}")
                nc.tensor.matmul(w3_ps[:], invn[:, BL - 1:BL], mmat,
                                 start=True, stop=True)
                w3_sb = sp.tile([1, PPIX], bf16, tag="w3sb", name=f"w3sb_{_rep}")
                nc.scalar.copy(w3_sb[:], w3_ps[:])
                wb_ps = ps.tile([128, PPIX], f32, tag="B", bufs=2,
                                name=f"wb_{_rep}")
                nc.tensor.matmul(wb_ps[:], bmh_sb[0:BL - 1, :],
                                 w_sb[0:BL - 1, :], start=True, stop=False)
                nc.tensor.matmul(wb_ps[:], bmh3_sb, w3_sb[:],
                                 start=False, stop=True)
                # wa = A * w  (+ fused v2 = rowsum)
                wa = ap_.tile([128, PPIX], bf16, tag="wa", name=f"wa_{_rep}")
                v2 = sp.tile([128, 1], f32, tag="v2", name=f"v2_{_rep}")
                nc.vector.scalar_tensor_tensor(
                    out=wa[:], in0=a_tF[:], scalar=1.0, in1=wb_ps[:],
                    op0=OP.mult, op1=OP.mult, accum_out=v2[:])

                # WA^T
                wat2 = []
                for q in range(2):
                    wt_ps = ps.tile([128, 128], bf16, tag="B", bufs=2,
                                    name=f"watps_{_rep}_{q}")
                    nc.tensor.transpose(wt_ps[:], wa[:, 128 * q:128 * q + 128],
                                        identb)
                    wq = gp.tile([128, 128], bf16, tag=f"wat{q}",
                                 name=f"wat_{_rep}_{q}")
                    nc.vector.tensor_copy(wq[:], wt_ps[:])
                    wat2.append(wq)

                # ============ v = sum_pix w*A*(y-c) ============
                v1_ps = ps.tile([128, 128], f32, tag="B", bufs=2,
                                name=f"v1_{_rep}")
                for b in range(BL):
                    for q in range(2):
                        nc.tensor.matmul(v1_ps[32 * b:32 * b + 32, :],
                                         wat2[q][:, 32 * b:32 * b + 32],
                                         yts[b][q][:],
                                         start=(q == 0), stop=(q == 1),
                                         tile_position=(0, 32 * b))
                v_sb = sp.tile([128, 128], bf16, tag="vsb", name=f"v_{_rep}")
                nc.vector.scalar_tensor_tensor(out=v_sb[:], in0=ctile,
                                               scalar=v2[:, 0:1], in1=v1_ps[:],
                                               op0=OP.mult, op1=OP.subtract)
                vt_ps = ps.tile([128, 128], bf16, tag="B", bufs=2,
                                name=f"vtps_{_rep}")
                nc.tensor.transpose(vt_ps[:], v_sb[:], identb)
                vt_sb = sp.tile([128, 128], bf16, tag="vt", name=f"vt_{_rep}")
                nc.scalar.copy(vt_sb[:], vt_ps[:])
                if WRITE_V:
                    nc.sync.dma_start(out=vout[:], in_=vt_sb[:])

            if STAGE >= 4:
                # ============ AllGather v ============
                v_loc = dp.tile([128, 128], bf16, tag="vloc", name=f"vloc_{_rep}")
                v_gth = dp.tile([NCORES * 128, 128], bf16, tag="vgth",
                                name=f"vgth_{_rep}", addr_space="Shared")
                nc.sync.dma_start(out=v_loc[:], in_=vt_sb[:])
                nc.gpsimd.collective_compute(
                    "AllGather", OP.bypass,
                    replica_groups=[list(range(NCORES))],
                    ins=[v_loc.opt()], outs=[v_gth.opt()])
                vall = yp.tile([128, NCORES, 128], bf16, tag="vall",
                               name=f"vall_{_rep}")
                nc.sync.dma_start(
                    out=vall[:], in_=v_gth.rearrange("(c d) n -> d c n", d=128))

                # ============ classifier (class shard) ============
                lg_ps = ps.tile([CSH, 32], f32, tag="B", bufs=2,
                                name=f"lg_{_rep}")
                vk = vall.rearrange("d c (b k) -> d c b k", k=32)
                for j in range(K):
                    nc.tensor.matmul(lg_ps[:], clsw_sb[:, j, :], vk[:, :, :, j],
                                     start=(j == 0), stop=(j == K - 1))
                lg_sb = sp.tile([CSH, 32], f32, tag="lg", name=f"lgsb_{_rep}")
                nc.scalar.activation(lg_sb[:], lg_ps[:], AF.Identity,
                                     bias=clsb_sb[:, 0:1])
                nc.sync.dma_start(out=logits[:], in_=lg_sb[:])

    nc.compile()
    return nc


def build_cls_nc():
    """Classifier-only kernel: replicated v_all input, per-core class shard."""
    import concourse.tile as tile
    from concourse import bacc, mybir
    f32 = mybir.dt.float32
    bf16 = mybir.dt.bfloat16
    AF = mybir.ActivationFunctionType
    nc = bacc.Bacc("TRN2", target_bir_lowering=False, debug=False,
                   num_devices=NCORES)
    vin = nc.dram_tensor("vin", [NCORES * 128, 128], bf16,
                         kind="ExternalInput").ap()
    clsw = nc.dram_tensor("clsw", [128, K * CSH], bf16,
                          kind="ExternalInput").ap()
    clsb = nc.dram_tensor("clsb", [CSH, 1], f32, kind="ExternalInput").ap()
    logits = nc.dram_tensor("logits", [CSH, B], f32, kind="ExternalOutput").ap()
    with tile.TileContext(nc) as tc, ExitStack() as ctx:
        cp = ctx.enter_context(tc.tile_pool(name="consts", bufs=1))
        ps = ctx.enter_context(tc.tile_pool(name="ps", bufs=1, space="PSUM"))
        clsw_sb = cp.tile([128, K, CSH], bf16)
        nc.gpsimd.dma_start(out=clsw_sb[:],
                          in_=clsw.rearrange("p (k n) -> p k n", k=K))
        clsb_sb = cp.tile([CSH, 1], f32)
        nc.gpsimd.dma_start(out=clsb_sb[:], in_=clsb[:])
        vall = cp.tile([128, NCORES, 128], bf16)
        nc.gpsimd.dma_start(out=vall[:],
                            in_=vin.rearrange("(c d) n -> d c n", d=128))
        lg_ps = ps.tile([CSH, 32], f32, tag="A", bufs=1)
        vk = vall.rearrange("d c (b k) -> d c b k", k=32)
        for j in range(K):
            nc.tensor.matmul(lg_ps[:], clsw_sb[:, j, :], vk[:, :, :, j],
                             start=(j == 0), stop=(j == K - 1))
        lg_sb = cp.tile([CSH, 32], f32)
        nc.scalar.activation(lg_sb[:], lg_ps[:], AF.Identity,
                             bias=clsb_sb[:, 0:1])
        nc.gpsimd.dma_start(out=logits[:], in_=lg_sb[:])
    nc.compile()
    return nc


# ----------------------------------------------------------------- host side
def make_inputs(x, conv_w, conv_b, bn_gamma, bn_beta, bn_mean, bn_var,
                codewords, scale, cls_w, cls_b):
    import ml_dtypes
    f = np.float32
    bf = ml_dtypes.bfloat16
    inv = (bn_gamma / np.sqrt(bn_var + BN_EPS)).astype(np.float64)
    wc = (conv_w.astype(np.float64) * inv[:, None]).astype(f)          # [D, CIN]
    biasd = ((conv_b - bn_mean).astype(np.float64) * inv + bn_beta).astype(f)
    c2 = (codewords.astype(np.float64) ** 2).sum(1).astype(f)          # [K]
    cw2 = (-2.0 * scale.astype(np.float64)[:, None]
           * codewords.astype(np.float64)).astype(f)                   # [K, D]

    # [128, NCH*128] partition-major conv weights
    wct = np.ascontiguousarray(
        wc.T.reshape(NCH, 128, 128).transpose(1, 0, 2).reshape(128, NCH * 128)
    ).astype(bf)

    # box matrix M [100, 256] (cols 196:256 zero)
    m = np.zeros((NP_, PPIX), f)
    for r in range(PH):
        for c in range(PW):
            for di in range(WIN):
                for dj in range(WIN):
                    m[r * PW + c, (r + di) * WW + (c + dj)] = 1.0
    mTp = np.ascontiguousarray(m.T)                                    # [256,100]

    pkb = np.zeros((128, PKB_N), f)
    pkb[:, PKB_CW2:PKB_CW2 + K] = cw2.T
    pkb[:, PKB_SC:PKB_SC + K] = np.broadcast_to(scale.astype(f), (128, K))
    pkb[:, PKB_C2T:PKB_C2T + K] = 2.0 * codewords.T
    pkb[:, PKB_ID:PKB_ID + 128] = np.eye(128, dtype=f)
    bmT = np.zeros((128, BL), f)
    for b in range(BL):
        bmT[32 * b:32 * b + 32, b] = 1.0
    pkb[:, PKB_ONE:PKB_ONE + K] = 1.0
    pkb[:, PKB_MMT:PKB_MMT + NP_] = mTp[0:128]
    pkb[:, PKB_MMT + NP_:PKB_MMT + 2 * NP_] = mTp[128:256]
    pkb[:, PKB_MMTH:PKB_MMTH + 2 * NP_] = \
        -0.5 * pkb[:, PKB_MMT:PKB_MMT + 2 * NP_]
    pkb[0:NP_, PKB_MM:PKB_MM + PPIX] = m
    bmf = np.ascontiguousarray(bmT.T)
    pkb[0:BL, PKB_BMH:PKB_BMH + 128] = bmf
    pkb[0:1, PKB_BMH3:PKB_BMH3 + 128] = bmf[3:4]
    pkb = pkb.astype(bf)

    pkf = np.zeros((128, PKF_N), f)
    pkf[:, PKF_BIAS] = biasd
    pkf[:, PKF_EB] = np.tile((scale * c2).astype(f), BL)
    pkf[:, PKF_NC2] = np.tile((-c2).astype(f), BL)
    pkf[:, PKF_CT:PKF_CT + 128] = np.tile(codewords.astype(f), (BL, 1))
    pkf[0:BL, PKF_BM:PKF_BM + 128] = bmf
    pkf[:, PKF_ONE:PKF_ONE + K] = 1.0

    cls_wT = np.ascontiguousarray(cls_w.T)                             # [4096, 1000]
    consts = dict(wct=wct, pkb=pkb, pkf=pkf)
    in_maps = []
    for c in range(NCORES):
        im = dict(consts)
        xs = np.ascontiguousarray(
            x[BL * c:BL * (c + 1)].reshape(BL, NCH, 128, NPIX)
            .transpose(0, 2, 1, 3).reshape(BL, 128, NCH * NPIX)).astype(bf)
        im["xin"] = xs
        sh = cls_wT[:, CSH * c:CSH * (c + 1)]                          # [4096, 125]
        im["clsw"] = np.ascontiguousarray(
            -sh.reshape(K, 128, CSH).transpose(1, 0, 2).reshape(128, K * CSH)
        ).astype(bf)
        pkfc = pkf.copy()
        pkfc[0:CSH, PKF_CLSB] = cls_b[CSH * c:CSH * (c + 1)]
        im["pkf"] = pkfc
        in_maps.append(im)
    return in_maps


def assemble(results):
    return np.concatenate([results[c]["logits"].T for c in range(NCORES)],
                          axis=1)


USE_COLLECTIVE = True


def kernel(**inputs):
    global STAGE, WRITE_V
    inputs = {k: np.asarray(v) for k, v in inputs.items()}
    from concourse.bass_utils import run_bass_kernel_spmd
    in_maps = make_inputs(**inputs)
    if USE_COLLECTIVE:
        key = "full"
        if key not in _NC_CACHE:
            STAGE, WRITE_V = 4, False
            _NC_CACHE[key] = build_nc(1)
        res = run_bass_kernel_spmd(_NC_CACHE[key], in_maps, list(range(NCORES)))
        return assemble(res.results)
    key = "v"
    if key not in _NC_CACHE:
        STAGE, WRITE_V = 3, True
        _NC_CACHE[key] = build_nc(1)
        _NC_CACHE["cls"] = build_cls_nc()
    res1 = run_bass_kernel_spmd(_NC_CACHE[key], in_maps, list(range(NCORES)))
    vall = np.concatenate([res1.results[c]["vout"] for c in range(NCORES)],
                          axis=0)
    clsb_full = np.asarray(inputs["cls_b"], np.float32)
    in_maps2 = [{"vin": vall, "clsw": in_maps[c]["clsw"],
                 "clsb": np.ascontiguousarray(
                     clsb_full[CSH * c:CSH * (c + 1)].reshape(CSH, 1))}
                for c in range(NCORES)]
    res2 = run_bass_kernel_spmd(_NC_CACHE["cls"], in_maps2, list(range(NCORES)))
    return assemble(res2.results)



# revision 52
# speedup vs baseline: 1.0862x; 1.0862x over previous
"""Trainium2 Bass kernel for nn_Att_patch_net (vq_codebook).

Math (per batch b):
  y[d,pix]   = relu(BN(conv1x1(x)))                    -- folded into Wc, bias_d
  dist[k,pix]= ||y[:,pix]-c[k]||^2 ; A = softmax_k(scale_k*dist)   (per PIXEL)
  R[pix]     = A[pix,k]*(y[d,pix]-c[k,d])  in R^{K x D}
  E[p]       = sum_{pix in patch p} R[pix]             -- 5x5 box sums (100 patches)
  v          = sum_p E[p]/||E[p]||  ;  logits = v @ cls_w.T + cls_b

Device strategy (8 cores, 4 batches each):
  * A[pix] depends only on the pixel -> per-pixel softmax via matmuls.
    u = x2 - dist = 2<y,c> - c2 computed DIRECTLY (no cancellation).
  * ||E[p]||^2 = sum_{pix,pix' in p} <R_pix,R_pix'> with the Gram matrix
      G = YY*S1 - (A'At2 + At2'A)/2    (bf16 matmuls, no R materialized)
    n2 = diag(M G M^T) via T' = (M G)^T pieces + fused mul-reduce.
  * v = sum_pix w[pix]*R[pix] with w = M^T (1/||E||)   (transposed box filter)
  * classifier: class-sharded (125/core) over an AllGather of v (bf16).

Precision: inputs/weights cast to bf16 host-side; PE accumulates fp32;
invn = exp(-0.5*ln(n2)) keeps every Act func in one act-table set.
"""
import numpy as np
from contextlib import ExitStack

B, CIN, HH, WW = 32, 2048, 14, 14
D, K, NCLASS = 128, 32, 1000
WIN = 5
PH = PW = 10
NP_ = 100          # patches
NPIX = HH * WW     # 196
PPIX = 200         # padded pixels per batch slot (gram layout)
CW = (128, PPIX - 128)  # pixel chunk widths (128 + 72)
NCORES = 8
BL = B // NCORES   # 4 local batches
CSH = NCLASS // NCORES  # 125 classes per core
NCH = CIN // 128   # 16 contraction chunks
NCH2 = CIN // 256  # 8 fp8 DoubleRow chunks (256-deep each)
BN_EPS = 1e-5
UNPIX = BL * NPIX  # 784 unpadded pixel columns

# packed bf16 const columns
PKB_CW2 = 0          # [128, 32]
PKB_C2T = 32         # [128, 32] 2*codewords^T (contiguous with CW2)
PKB_SC = 64          # [128, 32] scale bcast
PKB_ID = 96          # [128, 128] identity (bf16)
PKB_ONE = 224        # [128, 32] ones
PKB_MMT = 256        # [128, 200] M^T chunks (2x100)
PKB_MMTH = 456       # [128, 200] -0.5*M^T chunks
PKB_MM = 656         # [128, PPIX] M (rows 0:100)
PKB_BMH = 856        # [0:4, 128] bmh rows
PKB_BMH3 = 984       # [0:1, 128] bmh row 3 at partition 0
PKB_CLSB = 1112      # [0:1, 125] classifier bias row (per-core)
PKB_N = 1240
# packed f32 const columns
PKF_BIAS = 0
PKF_EB = 1
PKF_NC2 = 2
PKF_CT = 3           # [128, 128] codewords tiled
PKF_BM = 131         # [0:4, 128] bm rows
PKF_CLSB = 259       # [0:125, 1]
PKF_ONE = 260        # [128, 32] ones (f32)
PKF_N = 292

_NC_CACHE = {}
TTR = False
STAGE = 4           # 1=conv+softmax, 2=+gram/norms, 3=+v (no collective), 4=full
WRITE_V = False     # add a 'vout' external output with v^T (2-launch fallback)


# ----------------------------------------------------------------- device IR
def build_nc(n_rep: int = 1):
    import concourse.bass as bass
    import concourse.tile as tile
    from concourse import bacc, mybir

    f32 = mybir.dt.float32
    bf16 = mybir.dt.bfloat16
    fp8 = mybir.dt.float8e4
    AF = mybir.ActivationFunctionType
    OP = mybir.AluOpType
    AX = mybir.AxisListType
    DR = mybir.MatmulPerfMode.DoubleRow

    nc = bacc.Bacc("TRN2", target_bir_lowering=False, debug=False, num_devices=NCORES)

    def din(name, shape, dt=f32):
        return nc.dram_tensor(name, shape, dt, kind="ExternalInput").ap()

    xin = din("xin", [BL, 128, NCH2 * 2 * NPIX], fp8)
    wct = din("wct", [128, NCH2 * 2 * 128], fp8)
    pkb = din("pkb", [128, PKB_N], bf16)
    pkf = din("pkf", [128, PKF_N], f32)
    clsw = din("clsw", [128, K * CSH], bf16)
    logits = nc.dram_tensor("logits", [CSH, B], f32, kind="ExternalOutput").ap()
    vout = (nc.dram_tensor("vout", [128, 128], bf16, kind="ExternalOutput").ap()
            if WRITE_V else None)

    with tile.TileContext(nc) as tc, ExitStack() as ctx:
        cp = ctx.enter_context(tc.tile_pool(name="consts", bufs=1))
        xp = ctx.enter_context(tc.tile_pool(name="xp", bufs=4))
        yp = ctx.enter_context(tc.tile_pool(name="yp", bufs=2))
        ap_ = ctx.enter_context(tc.tile_pool(name="ap", bufs=2))
        sp = ctx.enter_context(tc.tile_pool(name="sp", bufs=2))
        gp = ctx.enter_context(tc.tile_pool(name="gp", bufs=2))
        ytp = ctx.enter_context(tc.tile_pool(name="ytp", bufs=10))
        ps = ctx.enter_context(tc.tile_pool(name="ps", bufs=1, space="PSUM"))
        dp = ctx.enter_context(tc.tile_pool(name="dp", bufs=2, space="DRAM"))

        # PE p-state warmup: keep PE busy from t=0 so the conv starts at
        # full clock. Zeros tile via memset (no DMA dependency); dead output.
        wz = cp.tile([128, 128], bf16)
        nc.vector.memset(wz[:], 0.0)
        wup = ps.tile([128, 128], bf16, tag="B", bufs=2, name="warm")
        for _i in range(26):
            nc.tensor.transpose(wup[:], wz[:], wz[:])
        # preload the one act-table set covering every func used here
        # (ln/exp/relu/identity/copy); the auto-inserter would otherwise
        # thrash natural_log <-> exp_and_others on the critical path.
        from concourse.hw_specs import get_activation_tables
        _set_id = list(get_activation_tables(nc.m.arch)).index(
            "natural_log_exp_and_others")
        nc.scalar.add_instruction(mybir.InstLoadActFuncSet(
            name=nc.get_next_instruction_name(), act_func_set_id=_set_id,
            ins=[], outs=[]))

        # ---- prologue DMAs, ordered for launch-critical-path overlap ----
        def issue_x(rep, b):
            xt = xp.tile([128, NCH2, 2, NPIX], fp8, tag="xg",
                         name=f"xt_{rep}_{b}", bufs=4)
            xv = xin[b].rearrange("p (n i m) -> p n i m", n=NCH2, i=2)
            h = NCH2 // 2
            nc.sync.dma_start(out=xt[:, 0:h, :, :], in_=xv[:, 0:h, :, :])
            nc.sync.dma_start(out=xt[:, h:NCH2, :, :], in_=xv[:, h:NCH2, :, :])
            return xt

        wc_sb = cp.tile([128, NCH2, 2, 128], fp8)
        wcv = wct.rearrange("p (n i m) -> p n i m", n=NCH2, i=2)
        nc.sync.dma_start(out=wc_sb[:], in_=wcv[:])
        x0 = [issue_x(0, 0)]
        pkb_sb = cp.tile([128, PKB_N], bf16)
        nc.sync.dma_start(out=pkb_sb[:], in_=pkb[:])
        pkf_sb = cp.tile([128, PKF_N], f32)
        nc.sync.dma_start(out=pkf_sb[:], in_=pkf[:])
        for _b in range(1, BL):
            x0.append(issue_x(0, _b))
        if STAGE >= 4:
            clsw_sb = cp.tile([128, K, CSH], bf16)
            nc.sync.dma_start(
                out=clsw_sb[:], in_=clsw.rearrange("p (k n) -> p k n", k=K))

        cw2T = pkb_sb[:, PKB_CW2:PKB_CW2 + K]
        scbc = pkb_sb[:, PKB_SC:PKB_SC + K]
        czT = pkb_sb[:, PKB_CW2:PKB_CW2 + 2 * K]
        identb = pkb_sb[:, PKB_ID:PKB_ID + 128]
        onesq = pkb_sb[0:32, PKB_ONE:PKB_ONE + 32]
        ones1 = pkb_sb[0:1, PKB_ONE:PKB_ONE + 32]
        clsbr = pkb_sb[0:1, PKB_CLSB:PKB_CLSB + CSH]
        mmT = pkb_sb[:, PKB_MMT:PKB_MMT + 2 * NP_].rearrange(
            "p (q n) -> p q n", q=2)
        mmTh = pkb_sb[:, PKB_MMTH:PKB_MMTH + 2 * NP_].rearrange(
            "p (q n) -> p q n", q=2)
        mmat = pkb_sb[0:NP_, PKB_MM:PKB_MM + PPIX]
        bmh_sb = pkb_sb[0:BL, PKB_BMH:PKB_BMH + 128]
        bmh3_sb = pkb_sb[0:1, PKB_BMH3:PKB_BMH3 + 128]
        biasd = pkf_sb[:, PKF_BIAS:PKF_BIAS + 1]
        ebias = pkf_sb[0:32, PKF_EB:PKF_EB + 1]
        negc2 = pkf_sb[0:32, PKF_NC2:PKF_NC2 + 1]
        ctile = pkf_sb[:, PKF_CT:PKF_CT + 128]
        clsb_sb = pkf_sb[0:CSH, PKF_CLSB:PKF_CLSB + 1]

        for _rep in range(n_rep):
            y_bf = yp.tile([128, BL, PPIX], bf16, tag="y", name=f"y_{_rep}")
            nc.vector.memset(y_bf[:, :, NPIX:PPIX], 0.0)
            a_tF = ap_.tile([128, PPIX], bf16, tag="at", name=f"at_{_rep}")
            nc.vector.memset(a_tF[:, NPIX:PPIX], 0.0)
            at2F = ap_.tile([128, PPIX], bf16, tag="at2", name=f"at2_{_rep}")
            nc.vector.memset(at2F[:, NPIX:PPIX], 0.0)
            u_sbF = sp.tile([128, NPIX], f32, tag="u", name=f"u_{_rep}")
            if STAGE >= 2:
                n2_sb = sp.tile([NP_, BL], f32, tag="n2", name=f"n2_{_rep}")
            yts = []

            def emit_front(b):
                # ============ conv + BN + ReLU (batch b) ============
                xt = x0[b] if _rep == 0 else issue_x(_rep, b)
                pc = ps.tile([128, NPIX], f32, tag="C", bufs=2,
                             name=f"pc_{_rep}_{b}", padded_shape=[128, 512])
                for k in range(NCH2):
                    nc.tensor.matmul(pc[:], wc_sb[:, k, :, :], xt[:, k, :, :],
                                     start=(k == 0), stop=(k == NCH2 - 1),
                                     perf_mode=DR)
                nc.scalar.activation(y_bf[:, b, 0:NPIX], pc[:], AF.Relu,
                                     bias=biasd)
                ysq_b = sp.tile([128, NPIX], bf16, tag="ysq", bufs=3,
                                name=f"ysq_{_rep}_{b}")
                nc.gpsimd.tensor_mul(ysq_b[:], y_bf[:, b, 0:NPIX],
                                     y_bf[:, b, 0:NPIX])

                # ============ per-pixel softmax (batch b) ============
                zu = ps.tile([64, NPIX], f32, tag="B", bufs=2,
                             name=f"zu_{_rep}_{b}", padded_shape=[64, 512])
                zb = zu[0:32, :]
                ub = zu[32:64, :]
                nc.tensor.matmul(zu[:], czT, y_bf[:, b, 0:NPIX], start=True,
                                 stop=False, skip_group_check=True)
                nc.tensor.matmul(zb, scbc, ysq_b[:], start=False, stop=True,
                                 skip_group_check=True)
                a_u = ap_.tile([32, NPIX], bf16, tag="au", bufs=3,
                               name=f"au_{_rep}_{b}")
                nc.scalar.activation(a_u[:], zb, AF.Exp, bias=ebias)
                u_sb = u_sbF[32 * b:32 * b + 32, :]
                nc.scalar.activation(u_sb[:], ub, AF.Identity, bias=negc2)
                # replicated-sum: all-ones [32,32] stationary gives the
                # softmax denominator already broadcast across partitions
                s_ps = ps.tile([32, NPIX], f32, tag="B", bufs=2,
                               name=f"s_{_rep}_{b}", padded_shape=[32, 512])
                nc.tensor.matmul(s_ps[:], onesq, a_u[:], start=True, stop=True)
                s_r = sp.tile([32, NPIX], f32, tag="sr", bufs=2,
                              name=f"sr_{_rep}_{b}")
                nc.vector.reciprocal(s_r[:], s_ps[:])
                asl = a_tF[32 * b:32 * b + 32, :]
                a2sl = at2F[32 * b:32 * b + 32, :]
                nc.vector.tensor_mul(asl[:, 0:NPIX], a_u[:], s_r[:])
                nc.gpsimd.tensor_mul(a2sl[:, 0:NPIX], u_sb[:], asl[:, 0:NPIX])
                if STAGE >= 3:
                    # y^T tiles for the final aggregation
                    ytb = []
                    for q in range(2):
                        w = CW[q]
                        ytq_ps = ps.tile([w, 128], bf16, tag="B", bufs=2,
                                         name=f"ytps_{_rep}_{b}_{q}")
                        nc.tensor.transpose(ytq_ps[:],
                                            y_bf[:, b, 128 * q:128 * q + w],
                                            identb)
                        ytq = ytp.tile([w, 128], bf16, tag="yt",
                                       name=f"yt_{_rep}_{b}_{q}")
                        if q == 0:
                            nc.scalar.copy(ytq[:], ytq_ps[:])
                        else:
                            nc.vector.tensor_copy(ytq[:], ytq_ps[:])
                        ytb.append(ytq)
                    yts.append(ytb)

            def emit_gram(b):
                asl = a_tF[32 * b:32 * b + 32, :]
                a2sl = at2F[32 * b:32 * b + 32, :]
                if STAGE >= 2:
                    # ============ Gram / patch norms (batch b) ============
                    g1s = []
                    wcs = []
                    for q in range(2):
                        yy_ps = ps.tile([128, PPIX], f32, tag="D", bufs=4,
                                        name=f"yy_{_rep}_{b}_{q}")
                        nc.tensor.matmul(yy_ps[:],
                                         y_bf[:, b, 128 * q:128 * q + 128],
                                         y_bf[:, b, :], start=True, stop=True)
                        s1_ps = ps.tile([128, PPIX], f32, tag="D", bufs=4,
                                        name=f"s1_{_rep}_{b}_{q}")
                        nc.tensor.matmul(s1_ps[:], asl[:, 128 * q:128 * q + 128],
                                         asl[:], start=True, stop=True,
                                         tile_position=(32 * b, 0))
                        w_ps = ps.tile([128, PPIX], f32, tag="D", bufs=4,
                                       name=f"w_{_rep}_{b}_{q}")
                        nc.tensor.matmul(w_ps[:], a2sl[:, 128 * q:128 * q + 128],
                                         asl[:], start=True, stop=False,
                                         tile_position=(32 * b, 0))
                        nc.tensor.matmul(w_ps[:], asl[:, 128 * q:128 * q + 128],
                                         a2sl[:], start=False, stop=True,
                                         tile_position=(32 * b, 0))
                        s1c = gp.tile([128, PPIX], bf16, tag=f"s1c{q}", bufs=3,
                                      name=f"s1c_{_rep}_{b}_{q}")
                        nc.scalar.copy(s1c[:], s1_ps[:])
                        g1 = gp.tile([128, PPIX], bf16, tag=f"g1{q}", bufs=3,
                                     name=f"g1_{_rep}_{b}_{q}")
                        nc.vector.tensor_mul(g1[:], yy_ps[:], s1c[:])
                        wc = gp.tile([128, PPIX], bf16, tag=f"wc{q}", bufs=3,
                                     name=f"wc_{_rep}_{b}_{q}")
                        if q == 0:
                            nc.scalar.copy(wc[:], w_ps[:])
                        else:
                            nc.vector.tensor_copy(wc[:], w_ps[:])
                        g1s.append(g1)
                        wcs.append(wc)
                    # T' = (M G)^T : [100, 256] psum, then n2 = rowsum(T'*M)
                    tp_ps = ps.tile([NP_, PPIX], f32, tag="D", bufs=4,
                                    name=f"tp_{_rep}_{b}")
                    for q in range(2):
                        nc.tensor.matmul(tp_ps[:], mmT[:, q, :], g1s[q][:],
                                         start=(q == 0), stop=False)
                        nc.tensor.matmul(tp_ps[:], mmTh[:, q, :], wcs[q][:],
                                         start=False, stop=(q == 1))
                    tm = sp.tile([NP_, PPIX], bf16, tag="tm",
                                 name=f"tm_{_rep}_{b}")
                    nc.vector.scalar_tensor_tensor(
                        out=tm[:], in0=tp_ps[:], scalar=1.0, in1=mmat,
                        op0=OP.mult, op1=OP.mult,
                        accum_out=n2_sb[:, b:b + 1])

            # software-pipelined emission: batch b's gram is emitted after
            # batch b+1's softmax so ready work never queues behind
            # not-yet-ready work on the in-order engines.
            for b in range(BL):
                emit_front(b)
                if b >= 1:
                    emit_gram(b - 1)
            emit_gram(BL - 1)

            if STAGE >= 3:
                # ============ patch weights: invn = |n2|^(-1/2) ========
                # batches 0-2 computed early (overlaps batch 3's gram);
                # batch 3 joins at the end. invn = exp(-0.5 ln(n2)) keeps
                # a single act-table set (n2 ~ 3e3..6e3, safely positive).
                invn = sp.tile([NP_, BL], bf16, tag="invn", name=f"invn_{_rep}")
                lnn = sp.tile([NP_, BL], f32, tag="lnn", name=f"lnn_{_rep}")
                nc.scalar.activation(lnn[:, 0:BL - 1], n2_sb[:, 0:BL - 1],
                                     AF.Ln)
                nc.scalar.activation(invn[:, 0:BL - 1], lnn[:, 0:BL - 1],
                                     AF.Exp, scale=-0.5)
                w_ps2 = ps.tile([BL, PPIX], f32, tag="B", bufs=2,
                                name=f"wps_{_rep}")
                nc.tensor.matmul(w_ps2[0:BL - 1, :], invn[:, 0:BL - 1], mmat,
                                 start=True, stop=True)
                w_sb = sp.tile([BL, PPIX], bf16, tag="wsb", name=f"wsb_{_rep}")
                nc.scalar.copy(w_sb[0:BL - 1, :], w_ps2[0:BL - 1, :])
                nc.scalar.activation(lnn[:, BL - 1:BL], n2_sb[:, BL - 1:BL],
                                     AF.Ln)
                nc.scalar.activation(invn[:, BL - 1:BL], lnn[:, BL - 1:BL],
                                     AF.Exp, scale=-0.5)
                w3_ps = ps.tile([1, PPIX], f32, tag="B", bufs=2,
                                name=f"w3ps_{_rep}")
                nc.tensor.matmul(w3_ps[:], invn[:, BL - 1:BL], mmat,
                                 start=True, stop=True)
                w3_sb = sp.tile([1, PPIX], bf16, tag="w3sb", name=f"w3sb_{_rep}")
                nc.scalar.copy(w3_sb[:], w3_ps[:])
                wb_ps = ps.tile([128, PPIX], f32, tag="B", bufs=2,
                                name=f"wb_{_rep}")
                nc.tensor.matmul(wb_ps[:], bmh_sb[0:BL - 1, :],
                                 w_sb[0:BL - 1, :], start=True, stop=False)
                nc.tensor.matmul(wb_ps[:], bmh3_sb, w3_sb[:],
                                 start=False, stop=True)
                # wa = A * w  (+ fused v2 = rowsum)
                wa = ap_.tile([128, PPIX], bf16, tag="wa", name=f"wa_{_rep}")
                v2 = sp.tile([128, 1], f32, tag="v2", name=f"v2_{_rep}")
                nc.vector.scalar_tensor_tensor(
                    out=wa[:], in0=a_tF[:], scalar=1.0, in1=wb_ps[:],
                    op0=OP.mult, op1=OP.mult, accum_out=v2[:])

                # WA^T
                wat2 = []
                for q in range(2):
                    w = CW[q]
                    wt_ps = ps.tile([w, 128], bf16, tag="B", bufs=2,
                                    name=f"watps_{_rep}_{q}")
                    nc.tensor.transpose(wt_ps[:], wa[:, 128 * q:128 * q + w],
                                        identb)
                    wq = gp.tile([w, 128], bf16, tag=f"wat{q}",
                                 name=f"wat_{_rep}_{q}")
                    nc.vector.tensor_copy(wq[:], wt_ps[:])
                    wat2.append(wq)

                # ============ v = sum_pix w*A*(y-c) ============
                v1_ps = ps.tile([128, 128], f32, tag="B", bufs=2,
                                name=f"v1_{_rep}")
                for b in range(BL):
                    for q in range(2):
                        nc.tensor.matmul(v1_ps[32 * b:32 * b + 32, :],
                                         wat2[q][:, 32 * b:32 * b + 32],
                                         yts[b][q][:],
                                         start=(q == 0), stop=(q == 1),
                                         tile_position=(0, 32 * b))
                v_sb = sp.tile([128, 128], bf16, tag="vsb", name=f"v_{_rep}")
                nc.vector.scalar_tensor_tensor(out=v_sb[:], in0=ctile,
                                               scalar=v2[:, 0:1], in1=v1_ps[:],
                                               op0=OP.mult, op1=OP.subtract)
                vt_ps = ps.tile([128, 128], bf16, tag="B", bufs=2,
                                name=f"vtps_{_rep}")
                nc.tensor.transpose(vt_ps[:], v_sb[:], identb)
                vt_sb = sp.tile([128, 128], bf16, tag="vt", name=f"vt_{_rep}")
                nc.scalar.copy(vt_sb[:], vt_ps[:])
                if WRITE_V:
                    nc.sync.dma_start(out=vout[:], in_=vt_sb[:])

            if STAGE >= 4:
                # ============ AllGather v ============
                v_loc = dp.tile([128, 128], bf16, tag="vloc", name=f"vloc_{_rep}")
                v_gth = dp.tile([NCORES * 128, 128], bf16, tag="vgth",
                                name=f"vgth_{_rep}", addr_space="Shared")
                nc.sync.dma_start(out=v_loc[:], in_=vt_sb[:])
                nc.gpsimd.collective_compute(
                    "AllGather", OP.bypass,
                    replica_groups=[list(range(NCORES))],
                    ins=[v_loc.opt()], outs=[v_gth.opt()])
                # gather-in split by source-core halves so the classifier
                # starts on cores 0-3 while cores 4-7 are still in flight
                vall = yp.tile([128, NCORES, 128], bf16, tag="vall",
                               name=f"vall_{_rep}")
                vgv = v_gth.rearrange("(c d) n -> d c n", d=128)
                hc = NCORES // 2
                nc.sync.dma_start(out=vall[:, 0:hc, :], in_=vgv[:, 0:hc, :])
                nc.sync.dma_start(out=vall[:, hc:, :], in_=vgv[:, hc:, :])

                # ============ classifier (class shard) ============
                # one start=True opens the whole [125, 32] zero region;
                # cls_b is accumulated as a rank-1 matmul
                lg_ps = ps.tile([CSH, 32], f32, tag="B", bufs=2,
                                name=f"lg_{_rep}")
                vk = vall.rearrange("d c (b k) -> d c b k", k=32)
                for h in range(2):
                    for j in range(K):
                        nc.tensor.matmul(
                            lg_ps[:, 16 * h:16 * h + 16],
                            clsw_sb[:, j, :], vk[:, hc * h:hc * h + hc, :, j],
                            start=(h == 0 and j == 0), stop=False,
                            skip_group_check=True)
                nc.tensor.matmul(lg_ps[:], clsbr, ones1, start=False,
                                 stop=True, skip_group_check=True)
                lg_sb = sp.tile([CSH, 32], f32, tag="lg", name=f"lgsb_{_rep}")
                nc.scalar.copy(lg_sb[:], lg_ps[:])
                nc.sync.dma_start(out=logits[:], in_=lg_sb[:])

    nc.compile()
    return nc


def build_cls_nc():
    """Classifier-only kernel: replicated v_all input, per-core class shard."""
    import concourse.tile as tile
    from concourse import bacc, mybir
    f32 = mybir.dt.float32
    bf16 = mybir.dt.bfloat16
    AF = mybir.ActivationFunctionType
    nc = bacc.Bacc("TRN2", target_bir_lowering=False, debug=False,
                   num_devices=NCORES)
    vin = nc.dram_tensor("vin", [NCORES * 128, 128], bf16,
                         kind="ExternalInput").ap()
    clsw = nc.dram_tensor("clsw", [128, K * CSH], bf16,
                          kind="ExternalInput").ap()
    clsb = nc.dram_tensor("clsb", [CSH, 1], f32, kind="ExternalInput").ap()
    logits = nc.dram_tensor("logits", [CSH, B], f32, kind="ExternalOutput").ap()
    with tile.TileContext(nc) as tc, ExitStack() as ctx:
        cp = ctx.enter_context(tc.tile_pool(name="consts", bufs=1))
        ps = ctx.enter_context(tc.tile_pool(name="ps", bufs=1, space="PSUM"))
        clsw_sb = cp.tile([128, K, CSH], bf16)
        nc.gpsimd.dma_start(out=clsw_sb[:],
                          in_=clsw.rearrange("p (k n) -> p k n", k=K))
        clsb_sb = cp.tile([CSH, 1], f32)
        nc.gpsimd.dma_start(out=clsb_sb[:], in_=clsb[:])
        vall = cp.tile([128, NCORES, 128], bf16)
        nc.gpsimd.dma_start(out=vall[:],
                            in_=vin.rearrange("(c d) n -> d c n", d=128))
        lg_ps = ps.tile([CSH, 32], f32, tag="A", bufs=1)
        vk = vall.rearrange("d c (b k) -> d c b k", k=32)
        for j in range(K):
            nc.tensor.matmul(lg_ps[:], clsw_sb[:, j, :], vk[:, :, :, j],
                             start=(j == 0), stop=(j == K - 1))
        lg_sb = cp.tile([CSH, 32], f32)
        nc.scalar.activation(lg_sb[:], lg_ps[:], AF.Identity,
                             bias=clsb_sb[:, 0:1])
        nc.gpsimd.dma_start(out=logits[:], in_=lg_sb[:])
    nc.compile()
    return nc


# ----------------------------------------------------------------- host side
def make_inputs(x, conv_w, conv_b, bn_gamma, bn_beta, bn_mean, bn_var,
                codewords, scale, cls_w, cls_b):
    import ml_dtypes
    f = np.float32
    bf = ml_dtypes.bfloat16
    f8 = ml_dtypes.float8_e4m3
    inv = (bn_gamma / np.sqrt(bn_var + BN_EPS)).astype(np.float64)
    wc = (conv_w.astype(np.float64) * inv[:, None]).astype(f)          # [D, CIN]
    biasd = ((conv_b - bn_mean).astype(np.float64) * inv + bn_beta).astype(f)
    c2 = (codewords.astype(np.float64) ** 2).sum(1).astype(f)          # [K]
    cw2 = (-2.0 * scale.astype(np.float64)[:, None]
           * codewords.astype(np.float64)).astype(f)                   # [K, D]

    # [128, NCH2*2*128] partition-major fp8 DoubleRow conv weights:
    # element (p, (n, i, d)) = wcT[c, d] with c = n*256 + i*128 + p
    wct = np.ascontiguousarray(
        wc.T.reshape(NCH2, 2, 128, 128).transpose(2, 0, 1, 3)
        .reshape(128, NCH2 * 2 * 128)
    ).astype(f8)

    # box matrix M [100, 256] (cols 196:256 zero)
    m = np.zeros((NP_, PPIX), f)
    for r in range(PH):
        for c in range(PW):
            for di in range(WIN):
                for dj in range(WIN):
                    m[r * PW + c, (r + di) * WW + (c + dj)] = 1.0
    mTp = np.ascontiguousarray(m.T)                                    # [256,100]

    pkb = np.zeros((128, PKB_N), f)
    pkb[:, PKB_CW2:PKB_CW2 + K] = cw2.T
    pkb[:, PKB_SC:PKB_SC + K] = np.broadcast_to(scale.astype(f), (128, K))
    pkb[:, PKB_C2T:PKB_C2T + K] = 2.0 * codewords.T
    pkb[:, PKB_ID:PKB_ID + 128] = np.eye(128, dtype=f)
    bmT = np.zeros((128, BL), f)
    for b in range(BL):
        bmT[32 * b:32 * b + 32, b] = 1.0
    pkb[:, PKB_ONE:PKB_ONE + K] = 1.0
    pkb[:, PKB_MMT:PKB_MMT + NP_] = mTp[0:128]
    pkb[:, PKB_MMT + NP_:PKB_MMT + 2 * NP_] = mTp[128:256]
    pkb[:, PKB_MMTH:PKB_MMTH + 2 * NP_] = \
        -0.5 * pkb[:, PKB_MMT:PKB_MMT + 2 * NP_]
    pkb[0:NP_, PKB_MM:PKB_MM + PPIX] = m
    bmf = np.ascontiguousarray(bmT.T)
    pkb[0:BL, PKB_BMH:PKB_BMH + 128] = bmf
    pkb[0:1, PKB_BMH3:PKB_BMH3 + 128] = bmf[3:4]
    pkb = pkb.astype(bf)

    pkf = np.zeros((128, PKF_N), f)
    pkf[:, PKF_BIAS] = biasd
    pkf[:, PKF_EB] = np.tile((scale * c2).astype(f), BL)
    pkf[:, PKF_NC2] = np.tile((-c2).astype(f), BL)
    pkf[:, PKF_CT:PKF_CT + 128] = np.tile(codewords.astype(f), (BL, 1))
    pkf[0:BL, PKF_BM:PKF_BM + 128] = bmf
    pkf[:, PKF_ONE:PKF_ONE + K] = 1.0

    cls_wT = np.ascontiguousarray(cls_w.T)                             # [4096, 1000]
    consts = dict(wct=wct, pkb=pkb, pkf=pkf)
    in_maps = []
    for c in range(NCORES):
        im = dict(consts)
        xs = np.ascontiguousarray(
            x[BL * c:BL * (c + 1)].reshape(BL, NCH2, 2, 128, NPIX)
            .transpose(0, 3, 1, 2, 4).reshape(BL, 128, NCH2 * 2 * NPIX)
        ).astype(f8)
        im["xin"] = xs
        sh = cls_wT[:, CSH * c:CSH * (c + 1)]                          # [4096, 125]
        im["clsw"] = np.ascontiguousarray(
            -sh.reshape(K, 128, CSH).transpose(1, 0, 2).reshape(128, K * CSH)
        ).astype(bf)
        pkbc = pkb.copy()
        pkbc[0:1, PKB_CLSB:PKB_CLSB + CSH] = \
            cls_b[CSH * c:CSH * (c + 1)].astype(bf)[None, :]
        im["pkb"] = pkbc
        in_maps.append(im)
    return in_maps


def assemble(results):
    return np.concatenate([results[c]["logits"].T for c in range(NCORES)],
                          axis=1)


USE_COLLECTIVE = True


def kernel(**inputs):
    global STAGE, WRITE_V
    inputs = {k: np.asarray(v) for k, v in inputs.items()}
    from concourse.bass_utils import run_bass_kernel_spmd
    in_maps = make_inputs(**inputs)
    if USE_COLLECTIVE:
        key = "full"
        if key not in _NC_CACHE:
            STAGE, WRITE_V = 4, False
            _NC_CACHE[key] = build_nc(1)
        res = run_bass_kernel_spmd(_NC_CACHE[key], in_maps, list(range(NCORES)))
        return assemble(res.results)
    key = "v"
    if key not in _NC_CACHE:
        STAGE, WRITE_V = 3, True
        _NC_CACHE[key] = build_nc(1)
        _NC_CACHE["cls"] = build_cls_nc()
    res1 = run_bass_kernel_spmd(_NC_CACHE[key], in_maps, list(range(NCORES)))
    vall = np.concatenate([res1.results[c]["vout"] for c in range(NCORES)],
                          axis=0)
    clsb_full = np.asarray(inputs["cls_b"], np.float32)
    in_maps2 = [{"vin": vall, "clsw": in_maps[c]["clsw"],
                 "clsb": np.ascontiguousarray(
                     clsb_full[CSH * c:CSH * (c + 1)].reshape(CSH, 1))}
                for c in range(NCORES)]
    res2 = run_bass_kernel_spmd(_NC_CACHE["cls"], in_maps2, list(range(NCORES)))
    return assemble(res2.results)



# revision 64
# speedup vs baseline: 1.1333x; 1.0433x over previous
"""Trainium2 Bass kernel for nn_Att_patch_net (vq_codebook).

Math (per batch b):
  y[d,pix]   = relu(BN(conv1x1(x)))                    -- folded into Wc, bias_d
  dist[k,pix]= ||y[:,pix]-c[k]||^2 ; A = softmax_k(scale_k*dist)   (per PIXEL)
  R[pix]     = A[pix,k]*(y[d,pix]-c[k,d])  in R^{K x D}
  E[p]       = sum_{pix in patch p} R[pix]             -- 5x5 box sums (100 patches)
  v          = sum_p E[p]/||E[p]||  ;  logits = v @ cls_w.T + cls_b

Device strategy (8 cores, 4 batches each):
  * A[pix] depends only on the pixel -> per-pixel softmax via matmuls.
    u = x2 - dist = 2<y,c> - c2 computed DIRECTLY (no cancellation).
  * ||E[p]||^2 = sum_{pix,pix' in p} <R_pix,R_pix'> with the Gram matrix
      G = YY*S1 - (A'At2 + At2'A)/2    (bf16 matmuls, no R materialized)
    n2 = diag(M G M^T) via T' = (M G)^T pieces + fused mul-reduce.
  * v = sum_pix w[pix]*R[pix] with w = M^T (1/||E||)   (transposed box filter)
  * classifier: class-sharded (125/core) over an AllGather of v (bf16).

Precision: inputs/weights cast to bf16 host-side; PE accumulates fp32;
invn = exp(-0.5*ln(n2)) keeps every Act func in one act-table set.
"""
import numpy as np
from contextlib import ExitStack

B, CIN, HH, WW = 32, 2048, 14, 14
D, K, NCLASS = 128, 32, 1000
WIN = 5
PH = PW = 10
NP_ = 100          # patches
NPIX = HH * WW     # 196
PPIX = 200         # padded pixels per batch slot (gram layout)
CW = (128, PPIX - 128)  # pixel chunk widths (128 + 72)
NCORES = 8
BL = B // NCORES   # 4 local batches
CSH = NCLASS // NCORES  # 125 classes per core
NCH = CIN // 128   # 16 contraction chunks
NCH2 = CIN // 256  # 8 fp8 DoubleRow chunks (256-deep each)
BN_EPS = 1e-5
UNPIX = BL * NPIX  # 784 unpadded pixel columns

# packed bf16 const columns
PKB_CW2 = 0          # [128, 32]
PKB_C2T = 32         # [128, 32] 2*codewords^T (contiguous with CW2)
PKB_SC = 64          # [128, 32] scale bcast
PKB_ID = 96          # [128, 128] identity (bf16)
PKB_ONE = 224        # [128, 32] ones
PKB_MMT = 256        # [128, 200] M^T chunks (2x100)
PKB_MMTH = 456       # [128, 200] -0.5*M^T chunks
PKB_MM = 656         # [128, PPIX] M (rows 0:100)
PKB_BMH = 856        # [0:4, 128] bmh rows
PKB_BMH3 = 984       # [0:1, 128] bmh row 3 at partition 0
PKB_CLSB = 1112      # [0:1, 125] classifier bias row (per-core)
PKB_N = 1240
# packed f32 const columns
PKF_BIAS = 0
PKF_EB = 1
PKF_NC2 = 2
PKF_CT = 3           # [128, 128] codewords tiled
PKF_BM = 131         # [0:4, 128] bm rows
PKF_CLSB = 259       # [0:125, 1]
PKF_ONE = 260        # [128, 32] ones (f32)
PKF_N = 292

_NC_CACHE = {}
TTR = False
STAGE = 4           # 1=conv+softmax, 2=+gram/norms, 3=+v (no collective), 4=full
WRITE_V = False     # add a 'vout' external output with v^T (2-launch fallback)


# ----------------------------------------------------------------- device IR
def build_nc(n_rep: int = 1):
    import concourse.bass as bass
    import concourse.tile as tile
    from concourse import bacc, mybir

    f32 = mybir.dt.float32
    bf16 = mybir.dt.bfloat16
    fp8 = mybir.dt.float8e4
    AF = mybir.ActivationFunctionType
    OP = mybir.AluOpType
    AX = mybir.AxisListType
    DR = mybir.MatmulPerfMode.DoubleRow

    nc = bacc.Bacc("TRN2", target_bir_lowering=False, debug=False, num_devices=NCORES)

    def din(name, shape, dt=f32):
        return nc.dram_tensor(name, shape, dt, kind="ExternalInput").ap()

    xin = din("xin", [BL, 128, NCH2 * 2 * NPIX], fp8)
    wct = din("wct", [128, NCH2 * 2 * 128], fp8)
    pkb = din("pkb", [128, PKB_N], bf16)
    pkf = din("pkf", [128, PKF_N], f32)
    clsw = din("clsw", [128, K * CSH], bf16)
    logits = nc.dram_tensor("logits", [CSH, B], f32, kind="ExternalOutput").ap()
    vout = (nc.dram_tensor("vout", [128, 128], bf16, kind="ExternalOutput").ap()
            if WRITE_V else None)

    with tile.TileContext(nc) as tc, ExitStack() as ctx:
        cp = ctx.enter_context(tc.tile_pool(name="consts", bufs=1))
        xp = ctx.enter_context(tc.tile_pool(name="xp", bufs=4))
        yp = ctx.enter_context(tc.tile_pool(name="yp", bufs=2))
        ap_ = ctx.enter_context(tc.tile_pool(name="ap", bufs=2))
        sp = ctx.enter_context(tc.tile_pool(name="sp", bufs=2))
        gp = ctx.enter_context(tc.tile_pool(name="gp", bufs=2))
        ytp = ctx.enter_context(tc.tile_pool(name="ytp", bufs=10))
        ps = ctx.enter_context(tc.tile_pool(name="ps", bufs=1, space="PSUM"))
        dp = ctx.enter_context(tc.tile_pool(name="dp", bufs=2, space="DRAM"))

        # PE p-state warmup: keep PE busy from t=0 so the conv starts at
        # full clock. Zeros tile via memset (no DMA dependency); dead output.
        wz = cp.tile([128, 128], bf16)
        nc.vector.memset(wz[:], 0.0)
        wup = ps.tile([128, 128], bf16, tag="B", bufs=2, name="warm")
        for _i in range(26):
            nc.tensor.transpose(wup[:], wz[:], wz[:])
        # preload the one act-table set covering every func used here
        # (ln/exp/relu/identity/copy); the auto-inserter would otherwise
        # thrash natural_log <-> exp_and_others on the critical path.
        from concourse.hw_specs import get_activation_tables
        _set_id = list(get_activation_tables(nc.m.arch)).index(
            "natural_log_exp_and_others")
        nc.scalar.add_instruction(mybir.InstLoadActFuncSet(
            name=nc.get_next_instruction_name(), act_func_set_id=_set_id,
            ins=[], outs=[]))

        # ---- prologue DMAs, ordered for launch-critical-path overlap ----
        def issue_x(rep, b, split=False):
            xt = xp.tile([128, NCH2, 2, NPIX], fp8, tag="xg",
                         name=f"xt_{rep}_{b}", bufs=4)
            xv = xin[b].rearrange("p (n i m) -> p n i m", n=NCH2, i=2)
            if split:
                h = NCH2 // 2
                nc.sync.dma_start(out=xt[:, 0:h, :, :], in_=xv[:, 0:h, :, :])
                nc.sync.dma_start(out=xt[:, h:NCH2, :, :],
                                  in_=xv[:, h:NCH2, :, :])
            else:
                nc.sync.dma_start(out=xt[:], in_=xv[:])
            return xt

        wc_sb = cp.tile([128, NCH2, 2, 128], fp8)
        wcv = wct.rearrange("p (n i m) -> p n i m", n=NCH2, i=2)
        nc.sync.dma_start(out=wc_sb[:], in_=wcv[:])
        x0 = [issue_x(0, 0, split=True)]
        pkb_sb = cp.tile([128, PKB_N], bf16)
        nc.sync.dma_start(out=pkb_sb[:], in_=pkb[:])
        pkf_sb = cp.tile([128, PKF_N], f32)
        nc.sync.dma_start(out=pkf_sb[:], in_=pkf[:])
        for _b in range(1, BL):
            x0.append(issue_x(0, _b))
        if STAGE >= 4:
            clsw_sb = cp.tile([128, K, CSH], bf16)
            nc.sync.dma_start(
                out=clsw_sb[:], in_=clsw.rearrange("p (k n) -> p k n", k=K))

        cw2T = pkb_sb[:, PKB_CW2:PKB_CW2 + K]
        scbc = pkb_sb[:, PKB_SC:PKB_SC + K]
        czT = pkb_sb[:, PKB_CW2:PKB_CW2 + 2 * K]
        identb = pkb_sb[:, PKB_ID:PKB_ID + 128]
        onesq = pkb_sb[0:32, PKB_ONE:PKB_ONE + 32]
        ones1 = pkb_sb[0:1, PKB_ONE:PKB_ONE + 32]
        clsbr = pkb_sb[0:1, PKB_CLSB:PKB_CLSB + CSH]
        mmT = pkb_sb[:, PKB_MMT:PKB_MMT + 2 * NP_].rearrange(
            "p (q n) -> p q n", q=2)
        mmTh = pkb_sb[:, PKB_MMTH:PKB_MMTH + 2 * NP_].rearrange(
            "p (q n) -> p q n", q=2)
        mmat = pkb_sb[0:NP_, PKB_MM:PKB_MM + PPIX]
        bmh_sb = pkb_sb[0:BL, PKB_BMH:PKB_BMH + 128]
        bmh3_sb = pkb_sb[0:1, PKB_BMH3:PKB_BMH3 + 128]
        biasd = pkf_sb[:, PKF_BIAS:PKF_BIAS + 1]
        ebias = pkf_sb[0:32, PKF_EB:PKF_EB + 1]
        negc2 = pkf_sb[0:32, PKF_NC2:PKF_NC2 + 1]
        ctile = pkf_sb[:, PKF_CT:PKF_CT + 128]
        clsb_sb = pkf_sb[0:CSH, PKF_CLSB:PKF_CLSB + 1]

        for _rep in range(n_rep):
            y_bf = yp.tile([128, BL, PPIX], bf16, tag="y", name=f"y_{_rep}")
            nc.vector.memset(y_bf[:, :, NPIX:PPIX], 0.0)
            a_tF = ap_.tile([128, PPIX], bf16, tag="at", name=f"at_{_rep}")
            nc.vector.memset(a_tF[:, NPIX:PPIX], 0.0)
            at2F = ap_.tile([128, PPIX], bf16, tag="at2", name=f"at2_{_rep}")
            nc.vector.memset(at2F[:, NPIX:PPIX], 0.0)
            u_sbF = sp.tile([128, NPIX], f32, tag="u", name=f"u_{_rep}")
            if STAGE >= 2:
                n2_sb = sp.tile([NP_, BL], f32, tag="n2", name=f"n2_{_rep}")
            yts = []

            def emit_front(b):
                # ============ conv + BN + ReLU (batch b) ============
                xt = x0[b] if _rep == 0 else issue_x(_rep, b)
                pc = ps.tile([128, NPIX], f32, tag="C", bufs=2,
                             name=f"pc_{_rep}_{b}", padded_shape=[128, 512])
                for k in range(NCH2):
                    nc.tensor.matmul(pc[:], wc_sb[:, k, :, :], xt[:, k, :, :],
                                     start=(k == 0), stop=(k == NCH2 - 1),
                                     perf_mode=DR)
                nc.scalar.activation(y_bf[:, b, 0:NPIX], pc[:], AF.Relu,
                                     bias=biasd)
                ysq_b = sp.tile([128, NPIX], bf16, tag="ysq", bufs=3,
                                name=f"ysq_{_rep}_{b}")
                nc.vector.tensor_mul(ysq_b[:], y_bf[:, b, 0:NPIX],
                                      y_bf[:, b, 0:NPIX])

                # ============ per-pixel softmax (batch b) ============
                zu = ps.tile([64, NPIX], f32, tag="B", bufs=2,
                             name=f"zu_{_rep}_{b}", padded_shape=[64, 512])
                zb = zu[0:32, :]
                ub = zu[32:64, :]
                nc.tensor.matmul(zu[:], czT, y_bf[:, b, 0:NPIX], start=True,
                                 stop=False, skip_group_check=True)
                nc.tensor.matmul(zb, scbc, ysq_b[:], start=False, stop=True,
                                 skip_group_check=True)
                a_u = ap_.tile([32, NPIX], bf16, tag="au", bufs=3,
                               name=f"au_{_rep}_{b}")
                nc.scalar.activation(a_u[:], zb, AF.Exp, bias=ebias)
                u_sb = u_sbF[32 * b:32 * b + 32, :]
                nc.scalar.activation(u_sb[:], ub, AF.Identity, bias=negc2)
                # replicated-sum: all-ones [32,32] stationary gives the
                # softmax denominator already broadcast across partitions
                s_ps = ps.tile([32, NPIX], f32, tag="B", bufs=2,
                               name=f"s_{_rep}_{b}", padded_shape=[32, 512])
                nc.tensor.matmul(s_ps[:], onesq, a_u[:], start=True, stop=True)
                s_r = sp.tile([32, NPIX], f32, tag="sr", bufs=2,
                              name=f"sr_{_rep}_{b}")
                nc.vector.reciprocal(s_r[:], s_ps[:])
                asl = a_tF[32 * b:32 * b + 32, :]
                a2sl = at2F[32 * b:32 * b + 32, :]
                nc.vector.tensor_mul(asl[:, 0:NPIX], a_u[:], s_r[:])
                nc.gpsimd.tensor_mul(a2sl[:, 0:NPIX], u_sb[:], asl[:, 0:NPIX])
                if STAGE >= 3:
                    # y^T tiles for the final aggregation
                    ytb = []
                    for q in range(2):
                        w = CW[q]
                        ytq_ps = ps.tile([w, 128], bf16, tag="B", bufs=2,
                                         name=f"ytps_{_rep}_{b}_{q}")
                        nc.tensor.transpose(ytq_ps[:],
                                            y_bf[:, b, 128 * q:128 * q + w],
                                            identb)
                        ytq = ytp.tile([w, 128], bf16, tag="yt",
                                       name=f"yt_{_rep}_{b}_{q}")
                        if q == 0:
                            nc.scalar.copy(ytq[:], ytq_ps[:])
                        else:
                            nc.vector.tensor_copy(ytq[:], ytq_ps[:])
                        ytb.append(ytq)
                    yts.append(ytb)

            def emit_gram(b):
                asl = a_tF[32 * b:32 * b + 32, :]
                a2sl = at2F[32 * b:32 * b + 32, :]
                if STAGE >= 2:
                    # ============ Gram / patch norms (batch b) ============
                    g1s = []
                    wcs = []
                    for q in range(2):
                        yy_ps = ps.tile([128, PPIX], f32, tag="D", bufs=4,
                                        name=f"yy_{_rep}_{b}_{q}")
                        nc.tensor.matmul(yy_ps[:],
                                         y_bf[:, b, 128 * q:128 * q + 128],
                                         y_bf[:, b, :], start=True, stop=True)
                        s1_ps = ps.tile([128, PPIX], f32, tag="D", bufs=4,
                                        name=f"s1_{_rep}_{b}_{q}")
                        nc.tensor.matmul(s1_ps[:], asl[:, 128 * q:128 * q + 128],
                                         asl[:], start=True, stop=True,
                                         tile_position=(32 * b, 0))
                        w_ps = ps.tile([128, PPIX], f32, tag="D", bufs=4,
                                       name=f"w_{_rep}_{b}_{q}")
                        nc.tensor.matmul(w_ps[:], a2sl[:, 128 * q:128 * q + 128],
                                         asl[:], start=True, stop=False,
                                         tile_position=(32 * b, 0))
                        nc.tensor.matmul(w_ps[:], asl[:, 128 * q:128 * q + 128],
                                         a2sl[:], start=False, stop=True,
                                         tile_position=(32 * b, 0))
                        s1c = gp.tile([128, PPIX], bf16, tag=f"s1c{q}", bufs=3,
                                      name=f"s1c_{_rep}_{b}_{q}")
                        nc.scalar.copy(s1c[:], s1_ps[:])
                        g1 = gp.tile([128, PPIX], bf16, tag=f"g1{q}", bufs=3,
                                     name=f"g1_{_rep}_{b}_{q}")
                        nc.vector.tensor_mul(g1[:], yy_ps[:], s1c[:])
                        wc = gp.tile([128, PPIX], bf16, tag=f"wc{q}", bufs=3,
                                     name=f"wc_{_rep}_{b}_{q}")
                        if q == 0:
                            nc.scalar.copy(wc[:], w_ps[:])
                        else:
                            nc.vector.tensor_copy(wc[:], w_ps[:])
                        g1s.append(g1)
                        wcs.append(wc)
                    # T' = (M G)^T : [100, 256] psum, then n2 = rowsum(T'*M)
                    tp_ps = ps.tile([NP_, PPIX], f32, tag="D", bufs=4,
                                    name=f"tp_{_rep}_{b}")
                    for q in range(2):
                        nc.tensor.matmul(tp_ps[:], mmT[:, q, :], g1s[q][:],
                                         start=(q == 0), stop=False)
                        nc.tensor.matmul(tp_ps[:], mmTh[:, q, :], wcs[q][:],
                                         start=False, stop=(q == 1))
                    tm = sp.tile([NP_, PPIX], bf16, tag="tm",
                                 name=f"tm_{_rep}_{b}")
                    nc.vector.scalar_tensor_tensor(
                        out=tm[:], in0=tp_ps[:], scalar=1.0, in1=mmat,
                        op0=OP.mult, op1=OP.mult,
                        accum_out=n2_sb[:, b:b + 1])

            # software-pipelined emission: batch b's gram is emitted after
            # batch b+1's softmax so ready work never queues behind
            # not-yet-ready work on the in-order engines.
            for b in range(BL):
                emit_front(b)
                if b >= 1:
                    emit_gram(b - 1)
            emit_gram(BL - 1)

            if STAGE >= 3:
                # ============ patch weights: invn = |n2|^(-1/2) ========
                # batches 0-2 computed early (overlaps batch 3's gram);
                # batch 3 joins at the end. invn = exp(-0.5 ln(n2)) keeps
                # a single act-table set (n2 ~ 3e3..6e3, safely positive).
                invn = sp.tile([NP_, BL], bf16, tag="invn", name=f"invn_{_rep}")
                lnn = sp.tile([NP_, BL], f32, tag="lnn", name=f"lnn_{_rep}")
                nc.scalar.activation(lnn[:, 0:BL - 1], n2_sb[:, 0:BL - 1],
                                     AF.Ln)
                nc.scalar.activation(invn[:, 0:BL - 1], lnn[:, 0:BL - 1],
                                     AF.Exp, scale=-0.5)
                w_ps2 = ps.tile([BL, PPIX], f32, tag="B", bufs=2,
                                name=f"wps_{_rep}")
                nc.tensor.matmul(w_ps2[0:BL - 1, :], invn[:, 0:BL - 1], mmat,
                                 start=True, stop=True)
                w_sb = sp.tile([BL, PPIX], bf16, tag="wsb", name=f"wsb_{_rep}")
                nc.scalar.copy(w_sb[0:BL - 1, :], w_ps2[0:BL - 1, :])
                nc.scalar.activation(lnn[:, BL - 1:BL], n2_sb[:, BL - 1:BL],
                                     AF.Ln)
                nc.scalar.activation(invn[:, BL - 1:BL], lnn[:, BL - 1:BL],
                                     AF.Exp, scale=-0.5)
                w3_ps = ps.tile([1, PPIX], f32, tag="B", bufs=2,
                                name=f"w3ps_{_rep}")
                nc.tensor.matmul(w3_ps[:], invn[:, BL - 1:BL], mmat,
                                 start=True, stop=True)
                w3_sb = sp.tile([1, PPIX], bf16, tag="w3sb", name=f"w3sb_{_rep}")
                nc.vector.tensor_copy(w3_sb[:], w3_ps[:])
                wb_ps = ps.tile([128, PPIX], f32, tag="B", bufs=2,
                                name=f"wb_{_rep}")
                nc.tensor.matmul(wb_ps[:], bmh_sb[0:BL - 1, :],
                                 w_sb[0:BL - 1, :], start=True, stop=False)
                nc.tensor.matmul(wb_ps[:], bmh3_sb, w3_sb[:],
                                 start=False, stop=True)
                # wa = A * w  (+ fused v2 = rowsum)
                wa = ap_.tile([128, PPIX], bf16, tag="wa", name=f"wa_{_rep}")
                v2 = sp.tile([128, 1], f32, tag="v2", name=f"v2_{_rep}")
                nc.vector.scalar_tensor_tensor(
                    out=wa[:], in0=a_tF[:], scalar=1.0, in1=wb_ps[:],
                    op0=OP.mult, op1=OP.mult, accum_out=v2[:])

                # WA^T
                wat2 = []
                for q in range(2):
                    w = CW[q]
                    wt_ps = ps.tile([w, 128], bf16, tag="B", bufs=2,
                                    name=f"watps_{_rep}_{q}")
                    nc.tensor.transpose(wt_ps[:], wa[:, 128 * q:128 * q + w],
                                        identb)
                    wq = gp.tile([w, 128], bf16, tag=f"wat{q}",
                                 name=f"wat_{_rep}_{q}")
                    nc.vector.tensor_copy(wq[:], wt_ps[:])
                    wat2.append(wq)

                # ============ v = sum_pix w*A*(y-c) ============
                v1_ps = ps.tile([128, 128], f32, tag="B", bufs=2,
                                name=f"v1_{_rep}")
                for b in range(BL):
                    for q in range(2):
                        nc.tensor.matmul(v1_ps[32 * b:32 * b + 32, :],
                                         wat2[q][:, 32 * b:32 * b + 32],
                                         yts[b][q][:],
                                         start=(q == 0), stop=(q == 1),
                                         tile_position=(0, 32 * b))
                v_sb = sp.tile([128, 128], bf16, tag="vsb", name=f"v_{_rep}")
                nc.vector.scalar_tensor_tensor(out=v_sb[:], in0=ctile,
                                               scalar=v2[:, 0:1], in1=v1_ps[:],
                                               op0=OP.mult, op1=OP.subtract)
                vt_ps = ps.tile([128, 128], bf16, tag="B", bufs=2,
                                name=f"vtps_{_rep}")
                nc.tensor.transpose(vt_ps[:], v_sb[:], identb)
                vt_sb = sp.tile([128, 128], bf16, tag="vt", name=f"vt_{_rep}")
                nc.vector.tensor_copy(vt_sb[:], vt_ps[:])
                if WRITE_V:
                    nc.sync.dma_start(out=vout[:], in_=vt_sb[:])

            if STAGE >= 4:
                # ============ AllGather v ============
                v_loc = dp.tile([128, 128], bf16, tag="vloc", name=f"vloc_{_rep}")
                v_gth = dp.tile([NCORES * 128, 128], bf16, tag="vgth",
                                name=f"vgth_{_rep}", addr_space="Shared")
                nc.sync.dma_start(out=v_loc[:], in_=vt_sb[:])
                nc.gpsimd.collective_compute(
                    "AllGather", OP.bypass,
                    replica_groups=[list(range(NCORES))],
                    ins=[v_loc.opt()], outs=[v_gth.opt()])
                # keep PE continuously busy through the collective wait so
                # the p-state stays at full clock for the classifier (dead
                # transposes of vt_sb; gated on v so they follow real work)
                wup2 = ps.tile([128, 128], bf16, tag="B", bufs=2,
                               name=f"warm2_{_rep}")
                for _i in range(480):
                    nc.tensor.transpose(wup2[:], vt_sb[:], identb)
                # gather-in split by source-core halves so the classifier
                # starts on cores 0-3 while cores 4-7 are still in flight
                vall = yp.tile([128, NCORES, 128], bf16, tag="vall",
                               name=f"vall_{_rep}")
                vgv = v_gth.rearrange("(c d) n -> d c n", d=128)
                hc = NCORES // 2
                nc.sync.dma_start(out=vall[:, 0:hc, :], in_=vgv[:, 0:hc, :])
                nc.sync.dma_start(out=vall[:, hc:, :], in_=vgv[:, hc:, :])

                # ============ classifier (class shard) ============
                # one start=True opens the whole [125, 32] zero region;
                # cls_b is accumulated as a rank-1 matmul
                lg_ps = ps.tile([CSH, 32], f32, tag="B", bufs=2,
                                name=f"lg_{_rep}")
                vk = vall.rearrange("d c (b k) -> d c b k", k=32)
                for h in range(2):
                    for j in range(K):
                        nc.tensor.matmul(
                            lg_ps[:, 16 * h:16 * h + 16],
                            clsw_sb[:, j, :], vk[:, hc * h:hc * h + hc, :, j],
                            start=(h == 0 and j == 0), stop=False,
                            skip_group_check=True)
                nc.tensor.matmul(lg_ps[:], clsbr, ones1, start=False,
                                 stop=True, skip_group_check=True)
                lg_sb = sp.tile([CSH, 32], f32, tag="lg", name=f"lgsb_{_rep}")
                nc.vector.tensor_copy(lg_sb[:], lg_ps[:])
                nc.sync.dma_start(out=logits[:], in_=lg_sb[:])

    nc.compile()
    return nc


def build_cls_nc():
    """Classifier-only kernel: replicated v_all input, per-core class shard."""
    import concourse.tile as tile
    from concourse import bacc, mybir
    f32 = mybir.dt.float32
    bf16 = mybir.dt.bfloat16
    AF = mybir.ActivationFunctionType
    nc = bacc.Bacc("TRN2", target_bir_lowering=False, debug=False,
                   num_devices=NCORES)
    vin = nc.dram_tensor("vin", [NCORES * 128, 128], bf16,
                         kind="ExternalInput").ap()
    clsw = nc.dram_tensor("clsw", [128, K * CSH], bf16,
                          kind="ExternalInput").ap()
    clsb = nc.dram_tensor("clsb", [CSH, 1], f32, kind="ExternalInput").ap()
    logits = nc.dram_tensor("logits", [CSH, B], f32, kind="ExternalOutput").ap()
    with tile.TileContext(nc) as tc, ExitStack() as ctx:
        cp = ctx.enter_context(tc.tile_pool(name="consts", bufs=1))
        ps = ctx.enter_context(tc.tile_pool(name="ps", bufs=1, space="PSUM"))
        clsw_sb = cp.tile([128, K, CSH], bf16)
        nc.gpsimd.dma_start(out=clsw_sb[:],
                          in_=clsw.rearrange("p (k n) -> p k n", k=K))
        clsb_sb = cp.tile([CSH, 1], f32)
        nc.gpsimd.dma_start(out=clsb_sb[:], in_=clsb[:])
        vall = cp.tile([128, NCORES, 128], bf16)
        nc.gpsimd.dma_start(out=vall[:],
                            in_=vin.rearrange("(c d) n -> d c n", d=128))
        lg_ps = ps.tile([CSH, 32], f32, tag="A", bufs=1)
        vk = vall.rearrange("d c (b k) -> d c b k", k=32)
        for j in range(K):
            nc.tensor.matmul(lg_ps[:], clsw_sb[:, j, :], vk[:, :, :, j],
                             start=(j == 0), stop=(j == K - 1))
        lg_sb = cp.tile([CSH, 32], f32)
        nc.scalar.activation(lg_sb[:], lg_ps[:], AF.Identity,
                             bias=clsb_sb[:, 0:1])
        nc.gpsimd.dma_start(out=logits[:], in_=lg_sb[:])
    nc.compile()
    return nc


# ----------------------------------------------------------------- host side
def make_inputs(x, conv_w, conv_b, bn_gamma, bn_beta, bn_mean, bn_var,
                codewords, scale, cls_w, cls_b):
    import ml_dtypes
    f = np.float32
    bf = ml_dtypes.bfloat16
    f8 = ml_dtypes.float8_e4m3
    inv = (bn_gamma / np.sqrt(bn_var + BN_EPS)).astype(np.float64)
    wc = (conv_w.astype(np.float64) * inv[:, None]).astype(f)          # [D, CIN]
    biasd = ((conv_b - bn_mean).astype(np.float64) * inv + bn_beta).astype(f)
    c2 = (codewords.astype(np.float64) ** 2).sum(1).astype(f)          # [K]
    cw2 = (-2.0 * scale.astype(np.float64)[:, None]
           * codewords.astype(np.float64)).astype(f)                   # [K, D]

    # [128, NCH2*2*128] partition-major fp8 DoubleRow conv weights:
    # element (p, (n, i, d)) = wcT[c, d] with c = n*256 + i*128 + p
    wct = np.ascontiguousarray(
        wc.T.reshape(NCH2, 2, 128, 128).transpose(2, 0, 1, 3)
        .reshape(128, NCH2 * 2 * 128)
    ).astype(f8)

    # box matrix M [100, 256] (cols 196:256 zero)
    m = np.zeros((NP_, PPIX), f)
    for r in range(PH):
        for c in range(PW):
            for di in range(WIN):
                for dj in range(WIN):
                    m[r * PW + c, (r + di) * WW + (c + dj)] = 1.0
    mTp = np.ascontiguousarray(m.T)                                    # [256,100]

    pkb = np.zeros((128, PKB_N), f)
    pkb[:, PKB_CW2:PKB_CW2 + K] = cw2.T
    pkb[:, PKB_SC:PKB_SC + K] = np.broadcast_to(scale.astype(f), (128, K))
    pkb[:, PKB_C2T:PKB_C2T + K] = 2.0 * codewords.T
    pkb[:, PKB_ID:PKB_ID + 128] = np.eye(128, dtype=f)
    bmT = np.zeros((128, BL), f)
    for b in range(BL):
        bmT[32 * b:32 * b + 32, b] = 1.0
    pkb[:, PKB_ONE:PKB_ONE + K] = 1.0
    pkb[:, PKB_MMT:PKB_MMT + NP_] = mTp[0:128]
    pkb[0:PPIX - 128, PKB_MMT + NP_:PKB_MMT + 2 * NP_] = mTp[128:PPIX]
    pkb[:, PKB_MMTH:PKB_MMTH + 2 * NP_] = \
        -0.5 * pkb[:, PKB_MMT:PKB_MMT + 2 * NP_]
    pkb[0:NP_, PKB_MM:PKB_MM + PPIX] = m
    bmf = np.ascontiguousarray(bmT.T)
    pkb[0:BL, PKB_BMH:PKB_BMH + 128] = bmf
    pkb[0:1, PKB_BMH3:PKB_BMH3 + 128] = bmf[3:4]
    pkb = pkb.astype(bf)

    pkf = np.zeros((128, PKF_N), f)
    pkf[:, PKF_BIAS] = biasd
    pkf[:, PKF_EB] = np.tile((scale * c2).astype(f), BL)
    pkf[:, PKF_NC2] = np.tile((-c2).astype(f), BL)
    pkf[:, PKF_CT:PKF_CT + 128] = np.tile(codewords.astype(f), (BL, 1))
    pkf[0:BL, PKF_BM:PKF_BM + 128] = bmf
    pkf[:, PKF_ONE:PKF_ONE + K] = 1.0

    cls_wT = np.ascontiguousarray(cls_w.T)                             # [4096, 1000]
    consts = dict(wct=wct, pkb=pkb, pkf=pkf)
    in_maps = []
    for c in range(NCORES):
        im = dict(consts)
        xs = np.ascontiguousarray(
            x[BL * c:BL * (c + 1)].reshape(BL, NCH2, 2, 128, NPIX)
            .transpose(0, 3, 1, 2, 4).reshape(BL, 128, NCH2 * 2 * NPIX)
        ).astype(f8)
        im["xin"] = xs
        sh = cls_wT[:, CSH * c:CSH * (c + 1)]                          # [4096, 125]
        im["clsw"] = np.ascontiguousarray(
            -sh.reshape(K, 128, CSH).transpose(1, 0, 2).reshape(128, K * CSH)
        ).astype(bf)
        pkbc = pkb.copy()
        pkbc[0:1, PKB_CLSB:PKB_CLSB + CSH] = \
            cls_b[CSH * c:CSH * (c + 1)].astype(bf)[None, :]
        im["pkb"] = pkbc
        in_maps.append(im)
    return in_maps


def assemble(results):
    return np.concatenate([results[c]["logits"].T for c in range(NCORES)],
                          axis=1)


USE_COLLECTIVE = True


def kernel(**inputs):
    global STAGE, WRITE_V
    inputs = {k: np.asarray(v) for k, v in inputs.items()}
    from concourse.bass_utils import run_bass_kernel_spmd
    in_maps = make_inputs(**inputs)
    if USE_COLLECTIVE:
        key = "full"
        if key not in _NC_CACHE:
            STAGE, WRITE_V = 4, False
            _NC_CACHE[key] = build_nc(1)
        res = run_bass_kernel_spmd(_NC_CACHE[key], in_maps, list(range(NCORES)))
        return assemble(res.results)
    key = "v"
    if key not in _NC_CACHE:
        STAGE, WRITE_V = 3, True
        _NC_CACHE[key] = build_nc(1)
        _NC_CACHE["cls"] = build_cls_nc()
    res1 = run_bass_kernel_spmd(_NC_CACHE[key], in_maps, list(range(NCORES)))
    vall = np.concatenate([res1.results[c]["vout"] for c in range(NCORES)],
                          axis=0)
    clsb_full = np.asarray(inputs["cls_b"], np.float32)
    in_maps2 = [{"vin": vall, "clsw": in_maps[c]["clsw"],
                 "clsb": np.ascontiguousarray(
                     clsb_full[CSH * c:CSH * (c + 1)].reshape(CSH, 1))}
                for c in range(NCORES)]
    res2 = run_bass_kernel_spmd(_NC_CACHE["cls"], in_maps2, list(range(NCORES)))
    return assemble(res2.results)

